# revision 52
# baseline (speedup 1.0000x reference)
"""Trainium2 Bass kernel for MLP-with-SOM-cosine-similarity (retrieval_knn).

Reference computation per (b, k) pair:
  ctx, ent: [L=128, D=128] slices of context[b, k, 0/1]
  sim[l, m] = cos(ctx[l], ent[m]); idx[l] = argmax_m sim[l, m]
  x = [ctx_n | ent_n[idx]] -> 6x tanh(Linear(256,256)) -> dot W_out -> sum over l
Output: [B=64, K=64] f32.

Strategy: data-parallel over batch dim (8 cores x 8 batches = 512 pairs/core).
Per 4-pair chunk on-device: row norms via Pool square/add-tree + DVE reduce
and Newton rsqrt on Pool; normalized rows materialized as bf16 hi + bf16 lo
(~16 mantissa bits), both pieces transposed feature-major on the DMA xbar
(dma_start_transpose is 2-byte only) and reassembled by a DVE add into fp32r
(11-bit) similarity operands — no PE transposes, no PSUM->SBUF f32 copies.
Similarity is an fp32r PE matmul with a 2-pair-wide rhs (free dim 256 -> 1
cyc/row, half of fp32's 4 cyc/row even counting the wasted half; bf16 would
flip ~0.3% of argmax picks -> 4.5e-2 rel err, over the 2e-2 budget; fp32r
flips ~105/524k -> 1.3e-2 measured on HW).  Argmax is reduce_max + is_equal
one-hot (exact fp32 compare), the one-hot is transposed on the DMA xbar, and
the gather is a one-hot matmul against bf16 normalized entities.  MLP runs in
bf16 with fp32 PSUM accumulation, tanh on ACT batched over 8 pairs (bias
fused).  The final dot sums over L first (Pool bf16->f32 add-tree + DVE
reduce, exact) then contracts with W_out in two tiny fp32 matmuls per block.

Engine streams are in-order, so emission order is the software pipeline: the
MLP of sub s-1 is emitted during sub s's window with the two 8-pair
supergroups' layers interleaved (A0 B0 A1 B1 ...) so each tanh hides behind
the sibling group's matmuls; the pre-stage phases (reassembly add -> sim ->
gather) are threaded between layers one slot apart so DVE/DMA latency hides
behind matmuls.  Window 0 is emitted express (4-pair chunks, no slot padding)
to cut the ACT ramp-in bubble.

Engine busy (CoreSim): ACT 798us (95%, binding), PE 742us, DVE 620us, Pool
588us, SP 489us -> ~834us/core vs 1533us for the v0 baseline.  ACT is at its
engine-rate floor (768 tanh insts of [128,1024] PSUM fp32); wider [128,2048]
activations would need 10 PSUM banks (mm 4x2 + sim/gather 2) > 8 available.
"""

from contextlib import ExitStack

import numpy as np
import ml_dtypes

import concourse.bacc as bacc
import concourse.tile as tile
from concourse import mybir
from concourse.alu_op_type import AluOpType
from concourse.bass_utils import run_bass_kernel_spmd
from concourse.masks import make_identity

BF16 = mybir.dt.bfloat16
F32 = mybir.dt.float32
F32R = mybir.dt.float32r
AF = mybir.ActivationFunctionType

B, K, L, D = 64, 64, 128, 128
N_CORES = 8
PAIRS = (B // N_CORES) * K          # 512 pairs per core
N_HIDDEN = 6
SUB = 16                            # pairs per DMA subgroup
GRP = 4                             # pairs per PSUM group
QRT = 4                             # pairs per norm-squares chunk
UNROLL = 128                        # pairs per output block

_cache = {}


def _build_bass(pairs=PAIRS, unroll=UNROLL):
    nc = bacc.Bacc("TRN2")

    ctx_dram = nc.dram_tensor("ctxpairs", [pairs, 2, L, D], F32, kind="ExternalInput")
    wt_dram = nc.dram_tensor("wt", [128, N_HIDDEN * 2 * 2 * 128], BF16, kind="ExternalInput")
    wout_dram = nc.dram_tensor("wout", [128, 2], F32, kind="ExternalInput")
    bias_dram = nc.dram_tensor("bias", [128, N_HIDDEN * 2], F32, kind="ExternalInput")
    bout_dram = nc.dram_tensor("bout", [1, 1], F32, kind="ExternalInput")
    out_dram = nc.dram_tensor("out", [1, pairs], F32, kind="ExternalOutput")

    n_sub = pairs // SUB
    subs_per_block = unroll // SUB

    with ExitStack() as ctx:
        tc = ctx.enter_context(tile.TileContext(nc))
        const = ctx.enter_context(tc.tile_pool(name="const", bufs=1))
        raw_pool = ctx.enter_context(tc.tile_pool(name="raw", bufs=10))
        trash_pool = ctx.enter_context(tc.tile_pool(name="trash", bufs=2))
        nrm_pool = ctx.enter_context(tc.tile_pool(name="nrm", bufs=2))
        tiny_pool = ctx.enter_context(tc.tile_pool(name="tiny", bufs=4))
        norm_sb = ctx.enter_context(tc.tile_pool(name="normsb", bufs=4))
        ent_pool = ctx.enter_context(tc.tile_pool(name="entsb", bufs=8))
        pre_sb = ctx.enter_context(tc.tile_pool(name="presb", bufs=4))
        x_pool = ctx.enter_context(tc.tile_pool(name="xsb", bufs=8))
        y_pool = ctx.enter_context(tc.tile_pool(name="ysb", bufs=6))
        s_pool = ctx.enter_context(tc.tile_pool(name="ssb", bufs=2))
        # PSUM: 8 banks total — scr(2) + mlp(6)
        ps_scr = ctx.enter_context(tc.tile_pool(name="psscr", bufs=2, space="PSUM"))
        ps_mlp = ctx.enter_context(tc.tile_pool(name="psmlp", bufs=3, space="PSUM"))

        wt_sb = const.tile([128, N_HIDDEN, 2, 2, 128], BF16)
        wout_sb = const.tile([128, 2], F32)
        bias_sb = const.tile([128, N_HIDDEN * 2], F32)
        bout_sb = const.tile([1, 1], F32)
        bout128 = const.tile([1, 1], F32)
        identb = const.tile([128, 128], BF16)

        def emit_consts():
            nc.sync.dma_start(out=wt_sb, in_=wt_dram.rearrange("a (i kc mc b) -> a i kc mc b", i=N_HIDDEN, kc=2, mc=2))
            nc.sync.dma_start(out=wout_sb, in_=wout_dram[:, :])
            nc.sync.dma_start(out=bias_sb, in_=bias_dram[:, :])
            nc.sync.dma_start(out=bout_sb, in_=bout_dram[:, :])
            nc.vector.tensor_scalar(out=bout128, in0=bout_sb, scalar1=float(L), scalar2=0.0,
                                    op0=AluOpType.mult, op1=AluOpType.add)
            make_identity(nc, identb)

        sub_state = {}   # sub index -> dict(raw, rinv, ctxn, entn, entnb, x_tiles)
        s_tiles = {}     # block index -> s_all tile

        def emit_load(s, q):
            # 4-pair DMA chunk (QRT-granular so the pipeline fills fast)
            if s not in sub_state:
                sub_state[s] = {"raw": [], "rinv": [], "x": [], "loT": [], "entnb": []}
            st = sub_state[s]
            raw = raw_pool.tile([128, QRT, 2, 128], F32, tag="raw")
            p0 = s * SUB + q * QRT
            nc.sync.dma_start(
                out=raw,
                in_=ctx_dram[p0 : p0 + QRT].rearrange("p c l d -> l p c d"),
            )
            st["raw"].append(raw)

        def emit_norms(s, q):
            st = sub_state[s]
            raw = st["raw"][q]
            nrm2 = nrm_pool.tile([128, 2 * QRT], F32, tag="nrm2")
            sq = trash_pool.tile([128, QRT, 2, 128], F32, tag="sq")
            nc.gpsimd.tensor_mul(sq, raw, raw)
            half = trash_pool.tile([128, QRT, 2, 64], F32, tag="half")
            nc.gpsimd.tensor_add(half, sq[:, :, :, 0:64], sq[:, :, :, 64:128])
            nc.vector.tensor_reduce(
                nrm2.rearrange("a (p c) -> a p c", p=QRT),
                half, axis=mybir.AxisListType.X, op=AluOpType.add,
            )
            nc.gpsimd.tensor_scalar(out=nrm2, in0=nrm2, scalar1=1.0 / 128.0,
                                    scalar2=0.0, op0=AluOpType.mult, op1=AluOpType.add)
            # rinv = 1/sqrt(nrm2*128) via Newton on x' = nrm2 ~ 1 (Pool)
            yv = tiny_pool.tile([128, 2 * QRT], F32, tag="newty")
            tv = tiny_pool.tile([128, 2 * QRT], F32, tag="newtt")
            nc.gpsimd.tensor_scalar(out=yv, in0=nrm2, scalar1=-0.5, scalar2=1.5,
                                    op0=AluOpType.mult, op1=AluOpType.add)
            for _ in range(3):
                nc.gpsimd.tensor_mul(tv, yv, yv)
                nc.gpsimd.tensor_mul(tv, tv, nrm2)
                nc.gpsimd.tensor_scalar(out=tv, in0=tv, scalar1=-0.5, scalar2=1.5,
                                        op0=AluOpType.mult, op1=AluOpType.add)
                nc.gpsimd.tensor_mul(yv, yv, tv)
            rinv = tiny_pool.tile([128, 2 * QRT], F32, tag="rinv")
            nc.gpsimd.tensor_scalar(out=rinv, in0=yv, scalar1=float(1.0 / np.sqrt(128.0)),
                                    scalar2=0.0, op0=AluOpType.mult, op1=AluOpType.add)
            st["rinv"].append(rinv)

        def emit_normalize(s, q):
            # Normalized rows in bf16 hi + bf16 lo (hi+lo carries ~16 mantissa
            # bits >= the 11 the fp32r similarity needs). Both pieces are
            # transposed on the DMA xbar (2-byte only) instead of the PE, and
            # reassembled feature-major by a DVE add -> no PE transposes, no
            # PSUM->SBUF f32 copies.  QRT == GRP so chunk q is GRP q.
            st = sub_state[s]
            raw, rinv = st["raw"][q], st["rinv"][q]
            ctxh = norm_sb.tile([128, QRT, 128], BF16, tag="ctxh")
            enth = ent_pool.tile([128, QRT, 128], BF16, tag="enth")
            ctxl = norm_sb.tile([128, QRT, 128], BF16, tag="ctxl")
            entl = norm_sb.tile([128, QRT, 128], BF16, tag="entl")
            for c, (h, lo) in enumerate(((ctxh, ctxl), (enth, entl))):
                ntmp = trash_pool.tile([128, QRT, 128], F32, tag="ntmp")
                for j in range(QRT):
                    nc.gpsimd.tensor_scalar_mul(ntmp[:, j, :], raw[:, j, c, :], rinv[:, 2 * j + c : 2 * j + c + 1])
                    nc.gpsimd.tensor_scalar_mul(h[:, j, :], raw[:, j, c, :], rinv[:, 2 * j + c : 2 * j + c + 1])
                nc.gpsimd.tensor_sub(lo, ntmp, h)
            # transposes of the four bf16 pieces; ctx-hi lands directly in the
            # x tile chunk0.  Steady state goes via the DMA xbar (free, but
            # ~3us DGE latency); window 0's first groups go via PE transposes
            # to shorten the ramp-in dependency chain.
            x_sb = x_pool.tile([128, 2, GRP, 128], BF16, tag="x")
            chT = x_sb[:, 0]
            clT = pre_sb.tile([128, GRP, 128], BF16, tag="clT")
            ehT = pre_sb.tile([128, GRP, 128], BF16, tag="ehT")
            elT = pre_sb.tile([128, GRP, 128], BF16, tag="elT")
            if s == 0 and q < 2:
                for src, dst in ((ctxh, chT), (ctxl, clT), (enth, ehT), (entl, elT)):
                    tp_ps = ps_scr.tile([128, GRP, 128], BF16, tag="scr")
                    for j in range(GRP):
                        nc.tensor.transpose(tp_ps[:, j, :], src[:, j, :], identb)
                    nc.vector.tensor_copy(dst, tp_ps)
            else:
                for j in range(GRP):
                    nc.sync.dma_start_transpose(out=chT[:, j, :], in_=ctxh[:, j, :])
                    nc.sync.dma_start_transpose(out=clT[:, j, :], in_=ctxl[:, j, :])
                    nc.sync.dma_start_transpose(out=ehT[:, j, :], in_=enth[:, j, :])
                    nc.sync.dma_start_transpose(out=elT[:, j, :], in_=entl[:, j, :])
            st["x"].append(x_sb)
            st["loT"].append((clT, ehT, elT))
            st["entnb"].append(enth)

        def emit_tp(s, q):
            # reassemble feature-major f32r similarity operands from the
            # DMA-transposed bf16 hi/lo pieces (DVE adds, exact in fp32)
            st = sub_state[s]
            clT, ehT, elT = st["loT"][q]
            chT = st["x"][q][:, 0]
            ctxnT = pre_sb.tile([128, GRP, 128], F32R, tag="ctxnT")
            entnT = pre_sb.tile([128, GRP, 128], F32R, tag="entnT")
            nc.vector.tensor_add(ctxnT, chT, clT)
            nc.vector.tensor_add(entnT, ehT, elT)
            st.setdefault("tp", []).append((ctxnT, entnT))

        def emit_sim(s, q):
            st = sub_state[s]
            ctxnT, entnT = st["tp"][q]
            # similarity: fp32r matmul with 2-pair-wide rhs (free dim 256 ->
            # 1 cyc/row, half the cost of fp32 at the price of a wasted half)
            mx = tiny_pool.tile([128, GRP], F32, tag="mx")
            oh = pre_sb.tile([128, GRP, 128], BF16, tag="oh")
            for jp in range(GRP // 2):
                gps = ps_scr.tile([128, 2, 256], F32, tag="scr")
                rhs2 = entnT[:, 2 * jp : 2 * jp + 2].rearrange("a j f -> a (j f)")
                for j in range(2):
                    nc.tensor.matmul(gps[:, j, :], lhsT=ctxnT[:, 2 * jp + j, :], rhs=rhs2)
                for j in range(2):
                    pj = 2 * jp + j
                    use = gps[:, j, j * 128 : (j + 1) * 128]
                    nc.vector.tensor_reduce(
                        mx[:, pj : pj + 1], use.unsqueeze(1),
                        axis=mybir.AxisListType.X, op=AluOpType.max,
                    )
                    nc.vector.tensor_tensor(
                        out=oh[:, pj, :], in0=use,
                        in1=mx[:, pj : pj + 1].broadcast_to([128, 128]),
                        op=AluOpType.is_equal,
                    )
            # transpose one-hot on the DMA xbar (SBUF->SBUF, off the PE/DVE);
            # first two groups go via PE to skip the DGE latency at ramp-in
            ohT = pre_sb.tile([128, GRP, 128], BF16, tag="ohTsb")
            if s == 0:
                ohT_ps = ps_scr.tile([128, GRP, 128], BF16, tag="scr")
                for j in range(GRP):
                    nc.tensor.transpose(ohT_ps[:, j, :], oh[:, j, :], identb)
                nc.vector.tensor_copy(ohT, ohT_ps)
            else:
                for j in range(GRP):
                    nc.sync.dma_start_transpose(out=ohT[:, j, :], in_=oh[:, j, :])
            st.setdefault("ohT", []).append(ohT)

        def emit_gather(s, q):
            st = sub_state[s]
            entnb = st["entnb"][q]
            ohT = st["ohT"][q]
            x_sb = st["x"][q]
            ch1 = ps_scr.tile([128, GRP, 128], F32, tag="scr")
            for j in range(GRP):
                nc.tensor.matmul(ch1[:, j, :], lhsT=entnb[:, j, :], rhs=ohT[:, j, :])
            nc.vector.tensor_copy(x_sb[:, 1], ch1)  # chunk1 bf16

        def mlp_sg_init(s, g2):
            st = sub_state[s]
            xt = st["x"]
            return {
                "s": s, "g2": g2,
                "xin": [[xt[2 * g2 + g][:, kc].rearrange("a g d -> a (g d)") for kc in range(2)]
                        for g in range(2)],
            }

        def emit_mlp_layer(sg, i):
            ya = y_pool.tile([128, 2, 2, GRP * 128], BF16, tag="y")
            for mc in range(2):
                mm = ps_mlp.tile([128, 2, GRP * 128], F32, tag="mm")
                for g in range(2):
                    nc.tensor.matmul(mm[:, g, :], lhsT=wt_sb[:, i, 0, mc, :],
                                     rhs=sg["xin"][g][0], start=True, stop=False)
                    nc.tensor.matmul(mm[:, g, :], lhsT=wt_sb[:, i, 1, mc, :],
                                     rhs=sg["xin"][g][1], start=False, stop=True)
                nc.scalar.activation(
                    out=ya[:, mc].rearrange("a g d -> a (g d)"),
                    in_=mm.rearrange("a g d -> a (g d)"),
                    func=AF.Tanh,
                    bias=bias_sb[:, 2 * i + mc : 2 * i + mc + 1],
                )
            sg["xin"] = [[ya[:, kc, g] for kc in range(2)] for g in range(2)]
            sg["ya"] = ya

        def emit_final(sg):
            # sum over L: Pool add-tree (exact, f32 accum) + DVE reduce
            s, g2 = sg["s"], sg["g2"]
            blk = s // subs_per_block
            ya4 = sg["ya"].rearrange("a kc g (p l) -> a kc g p l", p=GRP)
            t1 = trash_pool.tile([128, 2, 2, GRP, 64], F32, tag="t1")
            t2 = trash_pool.tile([128, 2, 2, GRP, 32], F32, tag="t2")
            col = (s % subs_per_block) * SUB + g2 * 2 * GRP
            # per-kc halves: each chain starts right after its own mc tanh
            for kcx in range(2):
                nc.gpsimd.tensor_add(t1[:, kcx], ya4[:, kcx, :, :, 0:64], ya4[:, kcx, :, :, 64:128])
                nc.gpsimd.tensor_add(t2[:, kcx], t1[:, kcx, :, :, 0:32], t1[:, kcx, :, :, 32:64])
                nc.vector.tensor_reduce(
                    s_tiles[blk][:, kcx, col : col + 2 * GRP].rearrange("a (g p) -> a g p", g=2),
                    t2[:, kcx], axis=mybir.AxisListType.X, op=AluOpType.add,
                )

        def emit_block_out(blk):
            # out[pair] = W_out . s_all[:, :, pair] (contract 256 = 2 chunks) + L*b_out
            s_all = s_tiles[blk]
            wo = ps_scr.tile([1, unroll], F32, tag="scr")
            nc.tensor.matmul(wo, lhsT=wout_sb[:, 0:1], rhs=s_all[:, 0], start=True, stop=False)
            nc.tensor.matmul(wo, lhsT=wout_sb[:, 1:2], rhs=s_all[:, 1], start=False, stop=True)
            res = nrm_pool.tile([1, unroll], F32, tag="res")
            nc.vector.tensor_scalar(out=res, in0=wo, scalar1=bout128[0:1, 0:1], scalar2=0.0,
                                    op0=AluOpType.add, op1=AluOpType.add)
            nc.sync.dma_start(out=out_dram[0:1, blk * unroll : (blk + 1) * unroll], in_=res)

        def release_sub(s):
            st = sub_state.pop(s, None)
            if st is not None:
                st.clear()

        # ---- main software-pipelined emission ----
        # Window s interleaves the pre-stage phases of sub s (tp-add -> sim
        # -> gather, one slot apart so DVE/DMA latency hides behind matmuls)
        # with the MLP layers of sub s-1's two supergroups (A i, B i pairs so
        # each tanh hides behind the sibling group's matmuls).
        # express window 0: QRT-granular chained emission, no slot padding,
        # so the first MLP (window 1) starts as early as possible; the first
        # raw chunks beat the weight DMAs onto the SP queue
        emit_load(0, 0)
        emit_load(0, 1)
        emit_consts()
        # interleave the first two chains so their Pool latencies overlap
        emit_norms(0, 0)
        emit_norms(0, 1)
        emit_normalize(0, 0)
        emit_normalize(0, 1)
        for q in range(2, SUB // QRT):
            emit_load(0, q)
            emit_norms(0, q)
            emit_normalize(0, q)
        # finish q0/q1 fully (PE-transposed, low latency) before q2/q3 so the
        # first supergroup's x tiles don't queue behind DMA-latency-gated sims
        emit_tp(0, 0)
        emit_tp(0, 1)
        emit_sim(0, 0)
        emit_gather(0, 0)
        emit_sim(0, 1)
        emit_gather(0, 1)
        emit_tp(0, 2)
        emit_tp(0, 3)
        emit_sim(0, 2)
        emit_gather(0, 2)
        emit_sim(0, 3)
        emit_gather(0, 3)
        prevA = prevB = None
        for s in range(n_sub):
            if s % subs_per_block == 0:
                sall = s_pool.tile([128, 2, unroll], F32, tag="sall")
                s_tiles[s // subs_per_block] = sall
            if s + 1 < n_sub:
                for q in range(SUB // QRT):
                    emit_load(s + 1, q)
            if prevA is None:
                chunks = []
            elif s == 1:
                # window 1: B(0)'s x tiles arrive late (its pre-chain fills
                # the pipeline), so let A(0) run ahead on its own layer chain
                # to keep ACT fed instead of strict A/B lockstep
                chunks = [(prevA, 0), (prevA, 1), (prevA, 2), (prevA, 3),
                          (prevB, 0), (prevA, 4), (prevB, 1), (prevA, 5),
                          (prevB, 2), (prevB, 3), (prevB, 4), (prevB, 5)]
            else:
                chunks = [(sg, i) for i in range(N_HIDDEN) for sg in (prevA, prevB)]
            for i in range(N_HIDDEN):
                if s > 0:
                    if i < SUB // GRP:
                        emit_tp(s, i)
                    if 1 <= i <= SUB // GRP:
                        emit_sim(s, i - 1)
                    if 2 <= i <= SUB // GRP + 1:
                        emit_gather(s, i - 2)
                for sg, li in chunks[2 * i : 2 * i + 2]:
                    emit_mlp_layer(sg, li)
                if 1 <= i <= SUB // QRT and s + 1 < n_sub:
                    # spread next sub's norms/normalize across slots so Pool
                    # finishes them before window s+1's reconstruction adds
                    emit_norms(s + 1, i - 1)
                    emit_normalize(s + 1, i - 1)
            if prevB is not None:
                emit_final(prevA)
                emit_final(prevB)
                if (s - 1) % subs_per_block == subs_per_block - 1:
                    emit_block_out((s - 1) // subs_per_block)
            release_sub(s - 1)
            prevA = mlp_sg_init(s, 0)
            prevB = mlp_sg_init(s, 1)
        # drain: MLP of the last sub
        for i in range(N_HIDDEN):
            emit_mlp_layer(prevA, i)
            emit_mlp_layer(prevB, i)
        emit_final(prevA)
        emit_final(prevB)
        emit_block_out((n_sub - 1) // subs_per_block)

    nc.compile()
    return nc


def _prep_weights(Ws, bs, W_out, b_out):
    Ws = np.asarray(Ws, dtype=np.float32)
    bs = np.asarray(bs, dtype=np.float32)
    W_out = np.asarray(W_out, dtype=np.float32)
    b_out = np.asarray(b_out, dtype=np.float32)
    # wt[a, i, kc, mc, b] = Ws[i, mc*128+b, kc*128+a]
    wt = np.transpose(
        Ws.reshape(N_HIDDEN, 2, 128, 2, 128),  # [i, mc, b, kc, a]
        (4, 0, 3, 1, 2),
    ).reshape(128, N_HIDDEN * 2 * 2 * 128)
    wt = np.ascontiguousarray(wt.astype(ml_dtypes.bfloat16))
    wout = np.ascontiguousarray(W_out.reshape(2, 128).T.astype(np.float32))
    bias = np.ascontiguousarray(
        np.transpose(bs.reshape(N_HIDDEN, 2, 128), (2, 0, 1)).reshape(128, N_HIDDEN * 2)
    ).astype(np.float32)
    bout = b_out.reshape(1, 1).astype(np.float32)
    return wt, wout, bias, bout


def make_in_maps(context, Ws, bs, W_out, b_out):
    context = np.ascontiguousarray(np.asarray(context, dtype=np.float32))
    wt, wout, bias, bout = _prep_weights(Ws, bs, W_out, b_out)
    shards = context.reshape(N_CORES, PAIRS, 2, L, D)
    return [
        {"ctxpairs": np.ascontiguousarray(shards[i]), "wt": wt, "wout": wout,
         "bias": bias, "bout": bout}
        for i in range(N_CORES)
    ]


def kernel(context, Ws, bs, W_out, b_out):
    in_maps = make_in_maps(context, Ws, bs, W_out, b_out)
    if "nc" not in _cache:
        _cache["nc"] = _build_bass()
    nc = _cache["nc"]
    r = run_bass_kernel_spmd(nc, in_maps, core_ids=list(range(N_CORES)))
    out = np.concatenate([r.results[i]["out"].reshape(B // N_CORES, K) for i in range(N_CORES)], axis=0)
    return out.astype(np.float32)


if __name__ == "__main__":
    import reference
    inputs = reference.setup_inputs()
    inputs = {k: np.asarray(v) for k, v in inputs.items()}
    expected = np.asarray(reference.reference(**inputs))
    actual = kernel(**inputs)
    err = np.linalg.norm(actual - expected) / np.linalg.norm(expected)
    print("Relative error:", err)
